# revision 1
# baseline (speedup 1.0000x reference)
"""Segment-max pooling (wordpiece->word) Bass kernel for TRN2, 8 cores.

Strategy: pure data parallel, 2 examples per core. Per example:
  - nonempty spans are split into pow2 length classes R in {1,2,4,8}
    (spans longer than RMAX=8 are chained through extra rows and
    max-combined on the host),
  - each class is sorted by length (desc) and packed into groups of
    <=128 lanes (one span per SBUF partition),
  - per group, `rnds` indirect DMA gathers pull the span tokens from
    the context table in HBM into disjoint slices of a [128,rnds,1024]
    SBUF tile (all gathers of a group run concurrently),
  - one strided in-place vector reduce_max folds the rounds axis into
    slice 0,
  - one plain DMA stores slice 0 to a per-group DRAM output tensor.
The host permutes group lanes back to span slots and assembles the
[B,S,D] zero-padded result.

Sync-wait budget: the walrus codegen used by the bass2jax/axon path
allows a single attached sync wait per instruction; _split_waits hoists
any extra Tile-generated waits into standalone EventSemaphore
instructions on the same engine queue. Per-group output tensors avoid
WAW serialization between stores.
"""

import sys

if "/opt/trn_rl_repo" not in sys.path:
    sys.path.insert(0, "/opt/trn_rl_repo")

import numpy as np

B, S, D, N = 16, 4096, 1024, 1024
NCORES = 8
EPC = B // NCORES  # examples per core
RMAX = 8
PAD_GIDX = 100000  # > EPC*S-1, within int32 after *D
CLASSES = (8, 4, 2)

_CACHE = {}
LAST_RESULTS = None


def _plan(spans):
    spans = np.asarray(spans).astype(np.int64)
    per_ex = []
    for b in range(B):
        st = spans[b, :, 0]
        ln = spans[b, :, 1] - st
        subs = {R: [] for R in CLASSES}
        fix = []  # (span_i, pooled_rows, direct_tokens) host combine entries
        chain = 0
        for i in np.nonzero(ln > 0)[0]:
            s = int(st[i])
            l = int(ln[i])
            if l == 1:
                # singleton span: the row is a verbatim context row; the
                # host fills it during assembly -- no device traffic
                fix.append((int(i), [], [s]))
            elif l <= RMAX:
                R = min(c for c in CLASSES if c >= l)
                subs[R].append((s, l, int(i)))
            else:
                rows = []
                toks = []
                for o in range(0, l, RMAX):
                    ls = min(RMAX, l - o)
                    if ls == 1:
                        toks.append(s + o)
                        continue
                    R = min(c for c in CLASSES if c >= ls)
                    row = N + chain
                    chain += 1
                    subs[R].append((s + o, ls, row))
                    rows.append(row)
                fix.append((int(i), rows, toks))
        for R in CLASSES:
            subs[R].sort(key=lambda t: -t[1])
        per_ex.append((subs, fix))

    calls = []  # static: (R, g, npg, rounds)
    for R in CLASSES:
        gmax = max(-(-len(p[0][R]) // 128) for p in per_ex)
        for g in range(gmax):
            npg = max(min(max(len(p[0][R]) - g * 128, 0), 128) for p in per_ex)
            rnds = max(
                (p[0][R][g * 128][1] if len(p[0][R]) > g * 128 else 0)
                for p in per_ex
            )
            if npg and rnds:
                calls.append((R, g, npg, rnds))
    gcols = sum(c[3] for c in calls)
    ngroups = len(calls)

    # pad lanes get an out-of-bounds index; the gather's bounds_check
    # silently skips them (no HBM traffic, lane ignored downstream)
    gidx = np.full((NCORES, 128, EPC * gcols), PAD_GIDX, np.int32)
    # host-side lane -> output row map per (example, group); -1 = pad
    lanemap = np.full((B, ngroups, 128), -1, np.int64)
    for b in range(B):
        c, e = divmod(b, EPC)
        subs = per_ex[b][0]
        col = e * gcols
        for gi, (R, g, npg, rnds) in enumerate(calls):
            lanes = subs[R][g * 128 : g * 128 + 128]
            for p, (s, l, row) in enumerate(lanes):
                lanemap[b, gi, p] = row
            for r in range(rnds):
                for p, (s, l, row) in enumerate(lanes):
                    gidx[c, p, col] = e * S + s + min(r, l - 1)
                col += 1
    fixups = [p[1] for p in per_ex]
    nchain = max((max((r for _, rows, _t in f for r in rows), default=N - 1) for f in fixups), default=N - 1) - N + 1
    sig = tuple(calls)
    return sig, calls, gcols, ngroups, gidx, lanemap, fixups, nchain


def _split_waits(nc):
    """Give every instruction at most one attached sync wait.

    The walrus codegen used by the bass2jax/axon path accepts a single
    sync-wait command per instruction, but Tile's add_semaphores may
    attach several (multiple DMA completion lanes, cross-engine deps).
    Semantics-preserving fix: keep one wait attached and hoist the rest
    into standalone InstEventSemaphore instructions inserted directly
    before the instruction on the same engine queue -- the sequencer
    executes them in order, so the wait set is unchanged.
    """
    from concourse import mybir

    # a sem id no instruction in the final program references (Tile
    # released its sems post-schedule, so the allocator would hand back
    # a live DMA-lane id)
    used = set()
    for bb in nc.main_func.blocks:
        for ins in bb.instructions:
            si = ins.sync_info
            if si is not None:
                for w in si.on_wait:
                    used.add(w.id)
                for u in si.on_update:
                    used.add(u.id)
    ws_id = max(used) + 1 if used else 0
    for bb in nc.main_func.blocks:
        insts = bb.instructions
        targets = []
        for pos, ins in enumerate(insts):
            si = ins.sync_info
            if si is not None and len(si.on_wait) > 1:
                targets.append((pos, ins))
        for pos, ins in reversed(targets):
            si = ins.sync_info
            waits = list(si.on_wait)
            keep = waits[-1]
            extra = waits[:-1]
            while len(si.on_wait) > 0:
                si.on_wait.pop()
            si.on_wait.append(keep)
            SyncInfo = type(si)
            SyncUpdate = type(si.on_update[0]) if si.on_update else None
            for k, w in enumerate(extra):
                ev = mybir.InstEventSemaphore(name=f"WS{k}-{ins.name}")
                ev.engine = ins.engine
                # the sim requires every executable instruction to have an
                # on_update; inc a dedicated sem nothing waits on
                upd = (
                    [
                        SyncUpdate(
                            sync_type="semaphore",
                            id=ws_id,
                            ant_name="ws_split",
                            update_mode="sem-inc",
                            update_value=1,
                        )
                    ]
                    if SyncUpdate is not None
                    else []
                )
                ev.sync_info = SyncInfo(on_wait=[w], on_update=upd)
                insts.insert(pos, ev)
                nc.inst_map[ev.name] = ev
    return nc


def _build(calls, gcols, ngroups):
    from concourse import bass, mybir, tile

    nc = bass.Bass()
    f32 = mybir.dt.float32
    i32 = mybir.dt.int32
    ctx_t = nc.declare_dram_parameter("ctx", [EPC * S, D], f32, isOutput=False)
    gidx_t = nc.declare_dram_parameter("gidx", [128, EPC * gcols], i32, isOutput=False)
    out_t = [
        nc.declare_dram_parameter(f"out{e}_{gi}", [128, D], f32, isOutput=True)
        for e in range(EPC)
        for gi in range(ngroups)
    ]
    colbase = []
    acc = 0
    for _R, _g, _npg, rnds in calls:
        colbase.append(acc)
        acc += rnds
    with tile.TileContext(nc) as tc:
        with (
            tc.tile_pool(name="sbuf", bufs=1) as pool,
            tc.tile_pool(name="scratch", bufs=3) as spool,
        ):
            nc.gpsimd.preamble()  # register init for bounds_check scalars
            breg = nc.gpsimd.to_reg(EPC * S - 1)  # shared bounds register
            gt = pool.tile([128, EPC * gcols], i32, tag="gidx")
            # per-example idx loads so example 0's gathers start sooner
            for e in range(EPC):
                nc.sync.dma_start(
                    out=gt[:, e * gcols : (e + 1) * gcols],
                    in_=gidx_t[:, e * gcols : (e + 1) * gcols],
                )
            # interleave the two examples' groups for tighter packing
            for gi, (R, g, npg, rnds) in enumerate(calls):
                for e in range(EPC):
                    col = e * gcols + colbase[gi]
                    # all rounds gather concurrently into disjoint slices
                    # of one wide tile; one strided in-place reduce folds
                    # them into slice 0
                    wide = spool.tile([128, rnds, D], f32, tag=f"w{R}")
                    for r in range(rnds):
                        nc.gpsimd.indirect_dma_start(
                            out=wide[0:npg, r, :],
                            out_offset=None,
                            in_=ctx_t[:],
                            in_offset=bass.IndirectOffsetOnAxis(
                                ap=gt[0:npg, col + r : col + r + 1], axis=0
                            ),
                            bounds_check=breg,
                            oob_is_err=False,
                        )
                    if rnds > 1:
                        nc.vector.reduce_max(
                            out=wide[0:npg, 0, :],
                            in_=wide[0:npg].transpose([0, 2, 1]),
                            axis=mybir.AxisListType.X,
                        )
                    # HWDGE store: keeps the big writes off the SWDGE
                    # track so they overlap the gathers
                    nc.sync.dma_start(
                        out=out_t[e * ngroups + gi][0:npg, :],
                        in_=wide[0:npg, 0, :],
                    )
    return _split_waits(nc)


def kernel(context, spans, trace=False):
    global LAST_RESULTS
    context = np.ascontiguousarray(np.asarray(context, dtype=np.float32))
    spans_np = np.asarray(spans)
    sig, calls, gcols, ngroups, gidx, lanemap, fixups, nchain = _plan(spans_np)
    if ngroups == 0:
        # every nonempty span is a singleton (or there are none):
        # assembly is purely host-side
        out = np.zeros((B, S, D), np.float32)
        for b in range(B):
            for i, rows, toks in fixups[b]:
                out[b, i] = context[b, toks].max(axis=0)
        return out
    if sig not in _CACHE:
        _CACHE[sig] = _build(calls, gcols, ngroups)
    nc = _CACHE[sig]

    from concourse.bass_utils import run_bass_kernel_spmd

    in_maps = [
        {
            "ctx": context[c * EPC : (c + 1) * EPC].reshape(EPC * S, D),
            "gidx": gidx[c],
        }
        for c in range(NCORES)
    ]
    LAST_RESULTS = run_bass_kernel_spmd(
        nc, in_maps, list(range(NCORES)), trace=trace
    )
    res = LAST_RESULTS.results

    out = np.zeros((B, S, D), np.float32)
    pooled = np.zeros((N + nchain, D), np.float32)
    for b in range(B):
        c, e = divmod(b, EPC)
        pooled[:] = 0.0
        for gi in range(ngroups):
            rows = lanemap[b, gi]
            valid = rows >= 0
            if valid.any():
                pooled[rows[valid]] = res[c][f"out{e}_{gi}"][: len(valid)][valid]
        out[b, :N] = pooled[:N]
        for i, rows, toks in fixups[b]:
            cands = []
            if rows:
                cands.append(pooled[rows].max(axis=0))
            if toks:
                cands.append(context[b, toks].max(axis=0))
            out[b, i] = cands[0] if len(cands) == 1 else np.maximum(cands[0], cands[1])
    return out



# revision 2
# speedup vs baseline: 2.2841x; 2.2841x over previous
"""Segment-max pooling (wordpiece->word) Bass kernel for TRN2, 8 cores.

Strategy: pure data parallel, 2 examples per core, fp16 on device.

Host planning (per core, both examples pooled together):
  - nonempty spans of length >= 2 become "lanes" (spans longer than
    RMAX=8 are split into <=RMAX chunks chained through extra rows and
    max-combined on the host; singleton spans/chunk-remainders are
    copied from fp32 context on the host -- no device traffic),
  - lanes are sorted by length (desc) and packed into groups of 128
    (one lane per SBUF partition). Group g needs rounds[g] = length of
    its longest lane gather rounds; sorted packing makes rounds[]
    non-increasing, so the groups active at round r are a prefix
    [0, G(r)).
  - columns are laid out ROUND-major: round r owns columns
    [cstart[r], cstart[r] + G(r)), so each round is ONE contiguous
    indirect gather across all its active groups (8 SWDGE instructions
    total instead of one per class/group/round -- the Pool engine's
    994ns-per-instruction SWDGE descriptor generation was a secondary
    bottleneck of the previous revision).

Device per round r:
  - one indirect DMA gathers row (start + min(r, len-1)) of every
    active lane from the fp16 context table into round r's column
    block (round 0 lands directly in the accumulator tile),
  - one DVE tensor_max folds the block into the accumulator prefix
    (fp16 gets the 2x_1p DVE mode),
  - groups whose last round is r (a suffix of the group list) are
    stored to a per-round DRAM output tensor right away, overlapping
    later gathers.

fp16 context halves the DMA traffic vs fp32; the cost is ~2^-11
relative rounding on pooled values. max() over fp16-rounded values is
exactly fp16(true max) (rounding is monotone), and the host patches
the few elements with |v| < 1e-5 from fp32 context, so per-element
relative error stays <= ~3e-3 even in the subnormal range.

Sync-wait budget: the walrus codegen used by the bass2jax/axon path
allows a single attached sync wait per instruction; _split_waits hoists
any extra Tile-generated waits into standalone EventSemaphore
instructions on the same engine queue.
"""

import sys

if "/opt/trn_rl_repo" not in sys.path:
    sys.path.insert(0, "/opt/trn_rl_repo")

import numpy as np

B, S, D, N = 16, 4096, 1024, 1024
NCORES = 8
EPC = B // NCORES  # examples per core
RMAX = 8
PAD_GIDX = 100000  # > EPC*S-1, within int32 after *D
TINY = 1e-5  # host-patch threshold for fp16 subnormal outputs

_CACHE = {}
LAST_RESULTS = None


def _plan(spans):
    spans = np.asarray(spans).astype(np.int64)
    # ---- per-example span triage ----------------------------------------
    # fixups[b]: list of (span_i, chain_rows, host_tokens)
    # lanes[c]:  list of (length, ex, start, b, row) sorted by length desc
    fixups = [[] for _ in range(B)]
    nchain = [0] * B
    lanes = [[] for _ in range(NCORES)]
    for b in range(B):
        c, e = divmod(b, EPC)
        st = spans[b, :, 0]
        ln = spans[b, :, 1] - st
        for i in np.nonzero(ln > 0)[0]:
            s = int(st[i])
            l = int(ln[i])
            if l == 1:
                fixups[b].append((int(i), [], [s]))
            elif l <= RMAX:
                lanes[c].append((l, e, s, b, int(i)))
            else:
                rows = []
                toks = []
                for o in range(0, l, RMAX):
                    ls = min(RMAX, l - o)
                    if ls == 1:
                        toks.append(s + o)
                        continue
                    row = N + nchain[b]
                    nchain[b] += 1
                    lanes[c].append((ls, e, s + o, b, row))
                    rows.append(row)
                fixups[b].append((int(i), rows, toks))
    for c in range(NCORES):
        lanes[c].sort(key=lambda t: -t[0])

    # ---- static cross-core group structure ------------------------------
    G0 = max(-(-len(lanes[c]) // 128) for c in range(NCORES))
    rounds = [
        max(
            (lanes[c][g * 128][0] if len(lanes[c]) > g * 128 else 0)
            for c in range(NCORES)
        )
        for g in range(G0)
    ]  # non-increasing
    Glist = []  # G(r) = #groups with rounds[g] > r, for r = 0..max_rounds-1
    for r in range(rounds[0] if rounds else 0):
        Glist.append(sum(1 for rg in rounds if rg > r))
    cstart = []
    acc_ = 0
    for G in Glist:
        cstart.append(acc_)
        acc_ += G
    CW = acc_
    # store ranges: after round r, groups [G(r+1), G(r)) are complete
    store_ranges = []  # (r, ga, gb)
    for r in range(len(Glist)):
        gb = Glist[r]
        ga = Glist[r + 1] if r + 1 < len(Glist) else 0
        if gb > ga:
            store_ranges.append((r, ga, gb))

    # ---- per-core gather indices + host lane map ------------------------
    gidx = np.full((NCORES, 128, max(CW, 1)), PAD_GIDX, np.int32)
    # lane -> (b, pooled-row); -1 = pad
    lane_b = np.full((NCORES, G0 * 128), -1, np.int64)
    lane_row = np.full((NCORES, G0 * 128), -1, np.int64)
    for c in range(NCORES):
        for j, (l, e, s, b, row) in enumerate(lanes[c]):
            g, p = divmod(j, 128)
            lane_b[c, j] = b
            lane_row[c, j] = row
            base = e * S + s
            for r in range(rounds[g]):
                gidx[c, p, cstart[r] + g] = base + min(r, l - 1)

    sig = (G0, CW, tuple(Glist), tuple(store_ranges))
    return sig, G0, CW, Glist, cstart, store_ranges, gidx, lane_b, lane_row, fixups, nchain


def _split_waits(nc):
    """Give every instruction at most one attached sync wait.

    The walrus codegen used by the bass2jax/axon path accepts a single
    sync-wait command per instruction, but Tile's add_semaphores may
    attach several (multiple DMA completion lanes, cross-engine deps).
    Semantics-preserving fix: keep one wait attached and hoist the rest
    into standalone InstEventSemaphore instructions inserted directly
    before the instruction on the same engine queue -- the sequencer
    executes them in order, so the wait set is unchanged.
    """
    from concourse import mybir

    used = set()
    for bb in nc.main_func.blocks:
        for ins in bb.instructions:
            si = ins.sync_info
            if si is not None:
                for w in si.on_wait:
                    used.add(w.id)
                for u in si.on_update:
                    used.add(u.id)
    ws_id = max(used) + 1 if used else 0
    for bb in nc.main_func.blocks:
        insts = bb.instructions
        targets = []
        for pos, ins in enumerate(insts):
            si = ins.sync_info
            if si is not None and len(si.on_wait) > 1:
                targets.append((pos, ins))
        for pos, ins in reversed(targets):
            si = ins.sync_info
            waits = list(si.on_wait)
            keep = waits[-1]
            extra = waits[:-1]
            while len(si.on_wait) > 0:
                si.on_wait.pop()
            si.on_wait.append(keep)
            SyncInfo = type(si)
            SyncUpdate = type(si.on_update[0]) if si.on_update else None
            for k, w in enumerate(extra):
                ev = mybir.InstEventSemaphore(name=f"WS{k}-{ins.name}")
                ev.engine = ins.engine
                upd = (
                    [
                        SyncUpdate(
                            sync_type="semaphore",
                            id=ws_id,
                            ant_name="ws_split",
                            update_mode="sem-inc",
                            update_value=1,
                        )
                    ]
                    if SyncUpdate is not None
                    else []
                )
                ev.sync_info = SyncInfo(on_wait=[w], on_update=upd)
                insts.insert(pos, ev)
                nc.inst_map[ev.name] = ev
    return nc


def _build(G0, CW, Glist, cstart, store_ranges):
    from concourse import bass, mybir, tile

    nc = bass.Bass()
    f16 = mybir.dt.float16
    i32 = mybir.dt.int32
    ctx_t = nc.declare_dram_parameter("ctx", [EPC * S, D], f16, isOutput=False)
    gidx_t = nc.declare_dram_parameter("gidx", [128, CW], i32, isOutput=False)
    out_t = {
        r: nc.declare_dram_parameter(f"out{r}", [128, (gb - ga) * D], f16, isOutput=True)
        for r, ga, gb in store_ranges
    }
    stores = {r: (ga, gb) for r, ga, gb in store_ranges}
    WW = CW - G0  # wide-tile columns (rounds >= 1)
    with tile.TileContext(nc) as tc:
        with tc.tile_pool(name="sbuf", bufs=1) as pool:
            nc.gpsimd.preamble()  # register init for bounds_check scalars
            breg = nc.gpsimd.to_reg(EPC * S - 1)
            gt = pool.tile([128, CW], i32, tag="gidx")
            acc = pool.tile([128, G0, D], f16, tag="acc")
            wide = pool.tile([128, max(WW, 1), D], f16, tag="wide")
            # split the idx load so round 0's gather starts sooner
            nc.sync.dma_start(out=gt[:, 0:G0], in_=gidx_t[:, 0:G0])
            if CW > G0:
                nc.sync.dma_start(out=gt[:, G0:CW], in_=gidx_t[:, G0:CW])
            for r in range(len(Glist)):
                G = Glist[r]
                if r == 0:
                    outap = acc[:, :, :]  # G(0) == G0
                else:
                    a = cstart[r] - G0
                    outap = wide[:, a : a + G, :]
                nc.gpsimd.indirect_dma_start(
                    out=outap,
                    out_offset=None,
                    in_=ctx_t[:],
                    in_offset=bass.IndirectOffsetOnAxis(
                        ap=gt[:, cstart[r] : cstart[r] + G], axis=0
                    ),
                    bounds_check=breg,
                    oob_is_err=False,
                )
                if r >= 1:
                    a = cstart[r] - G0
                    nc.vector.tensor_max(
                        out=acc[:, 0:G, :],
                        in0=acc[:, 0:G, :],
                        in1=wide[:, a : a + G, :],
                    )
                if r in stores:
                    ga, gb = stores[r]
                    nc.sync.dma_start(out=out_t[r][:, :], in_=acc[:, ga:gb, :])
    return _split_waits(nc)


def kernel(context, spans, trace=False):
    global LAST_RESULTS
    context = np.ascontiguousarray(np.asarray(context, dtype=np.float32))
    ctx16 = context.astype(np.float16)
    spans_np = np.asarray(spans)
    (
        sig,
        G0,
        CW,
        Glist,
        cstart,
        store_ranges,
        gidx,
        lane_b,
        lane_row,
        fixups,
        nchain,
    ) = _plan(spans_np)

    out = np.zeros((B, S, D), np.float32)
    maxchain = max(nchain) if max(nchain) else 0
    pooled = np.zeros((B, N + maxchain, D), np.float32)

    if G0 > 0:
        if sig not in _CACHE:
            _CACHE[sig] = _build(G0, CW, Glist, cstart, store_ranges)
        nc = _CACHE[sig]

        from concourse.bass_utils import run_bass_kernel_spmd

        in_maps = [
            {
                "ctx": ctx16[c * EPC : (c + 1) * EPC].reshape(EPC * S, D),
                "gidx": gidx[c],
            }
            for c in range(NCORES)
        ]
        LAST_RESULTS = run_bass_kernel_spmd(
            nc, in_maps, list(range(NCORES)), trace=trace
        )
        res = LAST_RESULTS.results

        for c in range(NCORES):
            # reassemble the accumulator: lane j = g*128 + p
            accv = np.zeros((128, G0, D), np.float16)
            for r, ga, gb in store_ranges:
                accv[:, ga:gb, :] = res[c][f"out{r}"].reshape(128, gb - ga, D)
            flat = accv.transpose(1, 0, 2).reshape(G0 * 128, D)
            valid = lane_b[c] >= 0
            pooled[lane_b[c][valid], lane_row[c][valid]] = flat[valid].astype(
                np.float32
            )

    for b in range(B):
        out[b, :N] = pooled[b, :N]
        for i, rows, toks in fixups[b]:
            cands = []
            if rows:
                cands.append(pooled[b, rows].max(axis=0))
            if toks:
                cands.append(context[b, toks].max(axis=0))
            out[b, i] = cands[0] if len(cands) == 1 else np.maximum(cands[0], cands[1])

    # fp16 subnormal patch: for device-pooled spans, recompute elements whose
    # magnitude is below TINY from the fp32 context (handful of elements).
    st = spans_np[..., 0].astype(np.int64)
    en = spans_np[..., 1].astype(np.int64)
    dev_span = (en - st) >= 2  # [B, N]
    cand = np.argwhere(dev_span[:, :, None] & (np.abs(out[:, :N]) < TINY))
    if len(cand):
        by_span = {}
        for b, i, d in cand:
            by_span.setdefault((b, i), []).append(d)
        for (b, i), ds in by_span.items():
            out[b, i, ds] = context[b, st[b, i] : en[b, i], ds].max(axis=0)
    return out


# revision 3
# speedup vs baseline: 2.2943x; 1.0045x over previous
"""Segment-max pooling (wordpiece->word) Bass kernel for TRN2, 8 cores.

Strategy: pure data parallel, 2 examples per core, fp16 on device.

Host planning (per core, both examples pooled together):
  - nonempty spans of length >= 2 become "lanes" (spans longer than
    RMAX=8 are split into <=RMAX chunks chained through extra rows and
    max-combined on the host; singleton spans/chunk remainders are
    copied from fp32 context on the host -- no device traffic),
  - lanes are sorted by length (desc) and packed into groups of 128
    (one lane per SBUF partition),
  - per group, a quantum Q <= min(lane length) is chosen and each
    indirect-DMA descriptor gathers Q CONSECUTIVE context rows
    (rows s+min(r*Q, len-Q) .. +Q stay inside the span; overlap
    re-reads are harmless for max). ceil(maxlen/Q) gather rounds cover
    the group, so a group of uniform length-8 chain lanes costs ONE
    SWDGE instruction instead of eight. Q trades DMA bytes
    (Q*ceil(L/Q) rows/lane) against Pool-engine SWDGE time
    (~1us/instruction); the planner enumerates Q per group.
  - the gather ucode only accepts single-column offset APs (one index
    per partition), hence one instruction per (group, round).

Device per group:
  - rounds[g] indirect gathers -> [128, rounds*Q, D] scratch slabs,
  - a DVE tensor_max fold tree halves the slab count per op (fp16 gets
    the 2x_1p DVE mode); the last fold writes the group's result slab,
  - a per-group store ships the result, overlapping later gathers.

fp16 context halves the DMA traffic vs fp32. max() over fp16-rounded
values is exactly fp16(true max) (rounding is monotone), and the host
patches the few elements with |v| < 1e-5 from fp32 context, so
per-element relative error stays <= ~3e-3 even in the subnormal range.

Sync-wait budget: the walrus codegen used by the bass2jax/axon path
allows a single attached sync wait per instruction; _split_waits hoists
any extra Tile-generated waits into standalone EventSemaphore
instructions on the same engine queue.
"""

import sys

if "/opt/trn_rl_repo" not in sys.path:
    sys.path.insert(0, "/opt/trn_rl_repo")

import numpy as np

B, S, D, N = 16, 4096, 1024, 1024
NCORES = 8
EPC = B // NCORES  # examples per core
RMAX = 8
PAD_GIDX = 100000  # > EPC*S-1, within int32 after *D
TINY = 1e-5  # host-patch threshold for fp16 subnormal outputs
POOL_W = 0.4  # SWDGE-instruction weight in the per-group quantum choice

_CACHE = {}
LAST_RESULTS = None


def _pick_q(L, m):
    """Quantum for a group with max lane length L, min lane length m."""
    best, best_cost = 1, None
    for q in range(1, m + 1):
        rnds = -(-L // q)
        cost = 728.0 * q * rnds + POOL_W * 1040.0 * rnds
        if best_cost is None or cost < best_cost or (cost == best_cost and q > best):
            best, best_cost = q, cost
    return best


def _plan(spans):
    spans = np.asarray(spans).astype(np.int64)
    # ---- per-example span triage ----------------------------------------
    fixups = [[] for _ in range(B)]  # (span_i, chain_rows, host_tokens)
    nchain = [0] * B
    lanes = [[] for _ in range(NCORES)]  # (length, ex, start, b, row)
    for b in range(B):
        c, e = divmod(b, EPC)
        st = spans[b, :, 0]
        ln = spans[b, :, 1] - st
        for i in np.nonzero(ln > 0)[0]:
            s = int(st[i])
            l = int(ln[i])
            if l == 1:
                fixups[b].append((int(i), [], [s]))
            elif l <= RMAX:
                lanes[c].append((l, e, s, b, int(i)))
            else:
                rows = []
                toks = []
                for o in range(0, l, RMAX):
                    ls = min(RMAX, l - o)
                    if ls == 1:
                        toks.append(s + o)
                        continue
                    row = N + nchain[b]
                    nchain[b] += 1
                    lanes[c].append((ls, e, s + o, b, row))
                    rows.append(row)
                fixups[b].append((int(i), rows, toks))
    for c in range(NCORES):
        lanes[c].sort(key=lambda t: -t[0])

    # ---- static cross-core group structure ------------------------------
    G0 = max(-(-len(lanes[c]) // 128) for c in range(NCORES))
    groups = []  # (Q, rounds, slabs, scroff)
    scroff = 0
    for g in range(G0):
        L = 0
        m = RMAX
        for c in range(NCORES):
            grp = lanes[c][g * 128 : (g + 1) * 128]
            if grp:
                L = max(L, grp[0][0])
                m = min(m, grp[-1][0])
        Q = _pick_q(L, m)
        rnds = -(-L // Q)
        groups.append((Q, rnds, rnds * Q, scroff))
        scroff += rnds * Q
    TOTSCR = scroff
    tot_instr = sum(gr[1] for gr in groups)

    # ---- per-core gather indices + host lane map ------------------------
    gidx = np.full((NCORES, 128, max(tot_instr, 1)), PAD_GIDX, np.int32)
    lane_b = np.full((NCORES, G0 * 128), -1, np.int64)
    lane_row = np.full((NCORES, G0 * 128), -1, np.int64)
    for c in range(NCORES):
        col = 0
        for g, (Q, rnds, slabs, off) in enumerate(groups):
            grp = lanes[c][g * 128 : (g + 1) * 128]
            for p, (l, e, s, b, row) in enumerate(grp):
                j = g * 128 + p
                lane_b[c, j] = b
                lane_row[c, j] = row
                for r in range(rnds):
                    gidx[c, p, col + r] = e * S + s + min(r * Q, l - Q)
            col += rnds

    sig = tuple(groups)
    return sig, G0, groups, TOTSCR, tot_instr, gidx, lane_b, lane_row, fixups, nchain


def _split_waits(nc):
    """Give every instruction at most one attached sync wait.

    The walrus codegen used by the bass2jax/axon path accepts a single
    sync-wait command per instruction, but Tile's add_semaphores may
    attach several (multiple DMA completion lanes, cross-engine deps).
    Semantics-preserving fix: keep one wait attached and hoist the rest
    into standalone InstEventSemaphore instructions inserted directly
    before the instruction on the same engine queue -- the sequencer
    executes them in order, so the wait set is unchanged.
    """
    from concourse import mybir

    used = set()
    for bb in nc.main_func.blocks:
        for ins in bb.instructions:
            si = ins.sync_info
            if si is not None:
                for w in si.on_wait:
                    used.add(w.id)
                for u in si.on_update:
                    used.add(u.id)
    ws_id = max(used) + 1 if used else 0
    for bb in nc.main_func.blocks:
        insts = bb.instructions
        targets = []
        for pos, ins in enumerate(insts):
            si = ins.sync_info
            if si is not None and len(si.on_wait) > 1:
                targets.append((pos, ins))
        for pos, ins in reversed(targets):
            si = ins.sync_info
            waits = list(si.on_wait)
            keep = waits[-1]
            extra = waits[:-1]
            while len(si.on_wait) > 0:
                si.on_wait.pop()
            si.on_wait.append(keep)
            SyncInfo = type(si)
            SyncUpdate = type(si.on_update[0]) if si.on_update else None
            for k, w in enumerate(extra):
                ev = mybir.InstEventSemaphore(name=f"WS{k}-{ins.name}")
                ev.engine = ins.engine
                upd = (
                    [
                        SyncUpdate(
                            sync_type="semaphore",
                            id=ws_id,
                            ant_name="ws_split",
                            update_mode="sem-inc",
                            update_value=1,
                        )
                    ]
                    if SyncUpdate is not None
                    else []
                )
                ev.sync_info = SyncInfo(on_wait=[w], on_update=upd)
                insts.insert(pos, ev)
                nc.inst_map[ev.name] = ev
    return nc


def _build(G0, groups, TOTSCR, tot_instr):
    from concourse import bass, mybir, tile

    nc = bass.Bass()
    f16 = mybir.dt.float16
    i32 = mybir.dt.int32
    ctx_t = nc.declare_dram_parameter("ctx", [EPC * S, D], f16, isOutput=False)
    gidx_t = nc.declare_dram_parameter("gidx", [128, tot_instr], i32, isOutput=False)
    out_t = [
        nc.declare_dram_parameter(f"out{g}", [128, D], f16, isOutput=True)
        for g in range(G0)
    ]
    with tile.TileContext(nc) as tc:
        with tc.tile_pool(name="sbuf", bufs=1) as pool:
            nc.gpsimd.preamble()  # register init for bounds_check scalars
            breg = nc.gpsimd.to_reg(EPC * S - 1)
            gt = pool.tile([128, tot_instr], i32, tag="gidx")
            res = pool.tile([128, G0, D], f16, tag="res")
            scr = pool.tile([128, TOTSCR, D], f16, tag="scr")
            nc.sync.dma_start(out=gt[:, :], in_=gidx_t[:, :])
            col = 0
            for g, (Q, rnds, slabs, off) in enumerate(groups):
                for r in range(rnds):
                    nc.gpsimd.indirect_dma_start(
                        out=scr[:, off + r * Q : off + (r + 1) * Q, :],
                        out_offset=None,
                        in_=ctx_t[:],
                        in_offset=bass.IndirectOffsetOnAxis(
                            ap=gt[:, col + r : col + r + 1], axis=0
                        ),
                        bounds_check=breg,
                        oob_is_err=False,
                    )
                col += rnds
                # fold tree: halve the live slab count per op; the final
                # fold writes the group's result slab
                n = slabs
                while n > 1:
                    h = n // 2
                    if n == 2:
                        nc.vector.tensor_max(
                            out=res[:, g, :],
                            in0=scr[:, off, :],
                            in1=scr[:, off + 1, :],
                        )
                    else:
                        nc.vector.tensor_max(
                            out=scr[:, off : off + h, :],
                            in0=scr[:, off : off + h, :],
                            in1=scr[:, off + n - h : off + n, :],
                        )
                    n -= h
                nc.sync.dma_start(out=out_t[g][:, :], in_=res[:, g, :])
    return _split_waits(nc)


def kernel(context, spans, trace=False):
    global LAST_RESULTS
    context = np.ascontiguousarray(np.asarray(context, dtype=np.float32))
    ctx16 = context.astype(np.float16)
    spans_np = np.asarray(spans)
    (
        sig,
        G0,
        groups,
        TOTSCR,
        tot_instr,
        gidx,
        lane_b,
        lane_row,
        fixups,
        nchain,
    ) = _plan(spans_np)

    out = np.zeros((B, S, D), np.float32)
    maxchain = max(nchain) if max(nchain) else 0
    pooled = np.zeros((B, N + maxchain, D), np.float32)

    if G0 > 0:
        if sig not in _CACHE:
            _CACHE[sig] = _build(G0, groups, TOTSCR, tot_instr)
        nc = _CACHE[sig]

        from concourse.bass_utils import run_bass_kernel_spmd

        in_maps = [
            {
                "ctx": ctx16[c * EPC : (c + 1) * EPC].reshape(EPC * S, D),
                "gidx": gidx[c],
            }
            for c in range(NCORES)
        ]
        LAST_RESULTS = run_bass_kernel_spmd(
            nc, in_maps, list(range(NCORES)), trace=trace
        )
        res = LAST_RESULTS.results

        for c in range(NCORES):
            resv = np.stack(
                [res[c][f"out{g}"] for g in range(G0)], axis=1
            )  # [128, G0, D]
            flat = resv.transpose(1, 0, 2).reshape(G0 * 128, D)
            valid = lane_b[c] >= 0
            pooled[lane_b[c][valid], lane_row[c][valid]] = flat[valid].astype(
                np.float32
            )

    for b in range(B):
        out[b, :N] = pooled[b, :N]
        for i, rows, toks in fixups[b]:
            cands = []
            if rows:
                cands.append(pooled[b, rows].max(axis=0))
            if toks:
                cands.append(context[b, toks].max(axis=0))
            out[b, i] = cands[0] if len(cands) == 1 else np.maximum(cands[0], cands[1])

    # fp16 subnormal patch: for device-pooled spans, recompute elements whose
    # magnitude is below TINY from the fp32 context (handful of elements).
    st = spans_np[..., 0].astype(np.int64)
    en = spans_np[..., 1].astype(np.int64)
    dev_span = (en - st) >= 2  # [B, N]
    cand = np.argwhere(dev_span[:, :, None] & (np.abs(out[:, :N]) < TINY))
    if len(cand):
        by_span = {}
        for b, i, d in cand:
            by_span.setdefault((b, i), []).append(d)
        for (b, i), ds in by_span.items():
            out[b, i, ds] = context[b, st[b, i] : en[b, i], ds].max(axis=0)
    return out
